# revision 3
# baseline (speedup 1.0000x reference)
"""Paged causal GQA attention on 8 TRN2 NeuronCores.

Problem (hardcoded): B=8 seqs x S=1024 tokens, H=32 q-heads, KVH=8 kv-heads
(GQA group 4), D=128, f32 in/out, paged KV cache (block_size 16, 512 blocks).

Strategy:
  - Host side: scatter k/v into the paged cache via slot_mapping and gather
    per-sequence K/V via block_tables (pure permutation / shard preparation,
    exactly the reference semantics). Then shard one sequence per core and
    pre-lay-out the operands for the device kernel: Q^T [H*D, S] and
    K^T [KVH*D, S] in bf16 (so the PE needs no on-device transposes), and
    V augmented with a ones-column per kv-head [S, KVH*(D+1)] bf16 (so the
    softmax denominator falls out of the PV matmul with no memsets).
  - Device side (per core, SPMD): causal GQA attention for one sequence.
    scores^T [k, q] = K^T-block-stationary matmuls; exp via Act engine with
    fused scale; P^T is directly the PV stationary operand. Scores are
    bank-packed into [128, 1536] PSUM tiles so each head needs only 4
    ACTIVATE instructions (Act engine is the bottleneck at ~123us of pure
    exp throughput). Causal diag masking via gpsimd affine_select post-exp.
    Output normalized on DVE (reciprocal + broadcast multiply), written
    bf16 and upcast on host.
  - All input staging DMAs are issued upfront on sync (inputs fit SBUF);
    output DMAs go on the vector queue so they never queue behind staging.
"""

import numpy as np

B, S, H, KVH, D = 8, 1024, 32, 8, 128
G = H // KVH
NB, BS = 512, 16
T = B * S
SCALE = 0.08838834764831845
NCORES = 8

_compiled = {}


def _build():
    import concourse.bass as bass
    import concourse.bacc as bacc
    import concourse.mybir as mybir
    import concourse.tile as tile

    f32 = mybir.dt.float32
    bf16 = mybir.dt.bfloat16
    EXP = mybir.ActivationFunctionType.Exp

    nc = bacc.Bacc("TRN2", target_bir_lowering=False, debug=False,
                   num_devices=NCORES)
    qtd = nc.dram_tensor("qt", [H * D, S], bf16, kind="ExternalInput").ap()
    ktd = nc.dram_tensor("kt", [KVH * D, S], bf16, kind="ExternalInput").ap()
    vad = nc.dram_tensor("va", [S, KVH * (D + 1)], bf16,
                         kind="ExternalInput").ap()
    od = nc.dram_tensor("out", [S, H * D], bf16, kind="ExternalOutput").ap()

    NT = S // 128            # 8 k/q tiles of 128
    CB = 4                   # q-blocks per chunk (chunk = 512 q cols)
    NCH = NT // CB           # chunks per head

    # Bank-packed score-tile layouts: per chunk, list of tiles; each tile is
    # a list of (k-block j, col offset). Every matmul stays inside a single
    # 512-col (2KB) PSUM bank. Offsets pack the 512/384/256/128-wide blocks
    # with no gaps so one ACTIVATE covers the whole tile.
    PACKS = {
        0: [[(0, 0), (1, 512), (3, 896), (2, 1024)]],                # 1280
        1: [[(0, 0), (1, 512), (2, 1024)],                            # 1536
            [(3, 0), (4, 512), (5, 1024), (7, 1408)],                 # 1536
            [(6, 0)]],                                                # 256
    }

    with tile.TileContext(nc) as tc:
        with (
            tc.tile_pool(name="ktp", bufs=KVH) as ktp,
            tc.tile_pool(name="qtp", bufs=KVH) as qtp,
            tc.tile_pool(name="vap", bufs=KVH) as vap,
            tc.tile_pool(name="pt", bufs=10) as ptp,
            tc.tile_pool(name="ost", bufs=3) as ostp,
            tc.tile_pool(name="small", bufs=4) as smallp,
            tc.tile_pool(name="psum_s", bufs=2, space="PSUM") as psum_s,
            tc.tile_pool(name="psum_o", bufs=2, space="PSUM") as psum_o,
        ):
            # ---- staging: all upfront, interleaved K/Q/V per group so the
            # first group's operands land first ----
            KTs, QTs, VAs = [], [], []
            for g in range(KVH):
                kt_t = ktp.tile([128, S], bf16, tag="kt")
                nc.sync.dma_start(kt_t[:], ktd[g * 128:(g + 1) * 128, :])
                qt_t = qtp.tile([128, G, S], bf16, tag="qt")
                nc.sync.dma_start(
                    qt_t[:], qtd[g * 512:(g + 1) * 512, :]
                    .rearrange("(h p) s -> p h s", p=128))
                va_t = vap.tile([128, NT, D + 1], bf16, tag="va")
                nc.sync.dma_start(
                    va_t[:], vad[:, g * 129:(g + 1) * 129]
                    .rearrange("(n p) c -> p n c", p=128))
                KTs.append(kt_t)
                QTs.append(qt_t)
                VAs.append(va_t)

            def qk_phase(KT, QT, c):
                # scores^T -> exp -> P^T tiles for one (chunk, head)
                i0 = c * CB
                pts = {}
                for pack in PACKS[c]:
                    st = psum_s.tile([128, 1536], f32, tag="st")
                    w = 0
                    for j, off in pack:
                        n = (CB - max(j - i0, 0)) * 128
                        qcol = max(j, i0) * 128
                        nc.tensor.matmul(
                            st[:, off:off + n],
                            lhsT=KT[:, j * 128:(j + 1) * 128],
                            rhs=QT[:, qcol:qcol + n],
                            start=True, stop=True,
                        )
                        w = max(w, off + n)
                    pt = ptp.tile([128, 1536], bf16, tag="pt")
                    nc.scalar.activation(pt[:, :w], st[:, :w], EXP,
                                         scale=SCALE)
                    for j, off in pack:
                        if j >= i0:
                            # zero strictly-lower (q < k) of diagonal block
                            nc.gpsimd.affine_select(
                                out=pt[:, off:off + 128],
                                in_=pt[:, off:off + 128],
                                compare_op=mybir.AluOpType.is_ge,
                                fill=0.0, base=0,
                                pattern=[[1, 128]],
                                channel_multiplier=-1,
                            )
                        pts[j] = (pt, off)
                return pts

            def pv_phase(VA, pts, ost, c, h4, g):
                # o blocks at col offsets ii*256, width D+1, split into two
                # 1-bank halves so each half frees as soon as its two blocks
                # are normalized; per-bank accumulation groups must not
                # interleave, so each block's start..stop runs to completion.
                i0 = c * CB
                for half in range(2):
                    o = psum_o.tile([128, 512], f32, tag="o")
                    for hi in range(2):
                        ii = half * 2 + hi
                        i = i0 + ii
                        for j in range(i + 1):
                            pt, off = pts[j]
                            col = off + (ii - max(j - i0, 0)) * 128
                            nc.tensor.matmul(
                                o[:, hi * 256: hi * 256 + D + 1],
                                lhsT=pt[:, col:col + 128],
                                rhs=VA[:, j, :],
                                start=(j == 0), stop=(j == i),
                            )
                    rec = smallp.tile([128, 2], f32, tag="rec")
                    nc.vector.reciprocal(rec[:], o[:, D::256])
                    ov = o[:].rearrange("p (b x) -> p b x", x=256)[:, :, 0:D]
                    rbc = (rec[:].rearrange("p b -> p b ()")
                           .broadcast_to((128, 2, D)))
                    nc.vector.tensor_tensor(
                        ost[:, half * 2:half * 2 + 2,
                            h4 * 128:(h4 + 1) * 128], ov, rbc,
                        mybir.AluOpType.mult)
                if h4 == G - 1:
                    nc.gpsimd.dma_start(
                        od[c * 512:(c + 1) * 512, g * 512:(g + 1) * 512]
                        .rearrange("(b p) d -> p b d", p=128),
                        ost[:],
                    )

            # ---- main loop, software-pipelined qk/pv phases ----
            pend = []
            for g in range(KVH):
                for c in range(NCH):
                    ost = ostp.tile([128, CB, G * D], bf16, tag="ost")
                    for h4 in range(G):
                        pts = qk_phase(KTs[g], QTs[g][:, h4, :], c)
                        pend.append((VAs[g], pts, ost, c, h4, g))
                        if len(pend) > 2:
                            pv_phase(*pend.pop(0))
            while pend:
                pv_phase(*pend.pop(0))

    nc.compile()
    return nc


def _get_nc():
    if "nc" not in _compiled:
        _compiled["nc"] = _build()
    return _compiled["nc"]


def kernel(q, k, v, k_cache, v_cache, slot_mapping, block_tables):
    import ml_dtypes
    from concourse.bass_utils import run_bass_kernel_spmd

    bf16 = ml_dtypes.bfloat16
    q = np.ascontiguousarray(np.asarray(q, dtype=np.float32))
    k = np.asarray(k, dtype=np.float32)
    v = np.asarray(v, dtype=np.float32)
    sm = np.asarray(slot_mapping).astype(np.int64)
    bt = np.asarray(block_tables).astype(np.int64)

    # store_kvcache + page gather (reference semantics, pure permutation)
    kc = np.asarray(k_cache, dtype=np.float32).reshape(NB * BS, KVH * D).copy()
    vc = np.asarray(v_cache, dtype=np.float32).reshape(NB * BS, KVH * D).copy()
    kc[sm] = k
    vc[sm] = v
    kg = kc.reshape(NB, BS, KVH * D)[bt].reshape(B, S, KVH * D)
    vg = vc.reshape(NB, BS, KVH * D)[bt].reshape(B, S, KVH * D)

    # per-core device layouts: Q^T, K^T, V+ones, all bf16
    qT = q.reshape(B, S, H * D).transpose(0, 2, 1).astype(bf16)   # [B,H*D,S]
    kT = kg.transpose(0, 2, 1).astype(bf16)                       # [B,KVH*D,S]
    va = np.ones((B, S, KVH, D + 1), dtype=np.float32)
    va[:, :, :, :D] = vg.reshape(B, S, KVH, D)
    va = va.reshape(B, S, KVH * (D + 1)).astype(bf16)

    in_maps = [
        {"qt": np.ascontiguousarray(qT[i]),
         "kt": np.ascontiguousarray(kT[i]),
         "va": np.ascontiguousarray(va[i])}
        for i in range(NCORES)
    ]
    nc = _get_nc()
    res = run_bass_kernel_spmd(nc, in_maps, core_ids=list(range(NCORES)))
    _compiled["last_result"] = res
    out = np.concatenate([res.results[i]["out"] for i in range(NCORES)],
                         axis=0)
    return out.astype(np.float32)


# revision 6
# speedup vs baseline: 1.3084x; 1.3084x over previous
"""Paged causal GQA attention on 8 TRN2 NeuronCores.

Problem (hardcoded): B=8 seqs x S=1024 tokens, H=32 q-heads, KVH=8 kv-heads
(GQA group 4), D=128, f32 in/out, paged KV cache (block_size 16, 512 blocks).

Strategy:
  - Host side: scatter k/v into the paged cache via slot_mapping and gather
    per-sequence K/V via block_tables (pure permutation / shard preparation,
    exactly the reference semantics). Then shard one sequence per core and
    pre-lay-out the operands for the device kernel: Q^T [H*D, S] and
    K^T [KVH*D, S] in bf16 (so the PE needs no on-device transposes), and
    V augmented with a ones-column per kv-head [S, KVH*(D+1)] bf16 (so the
    softmax denominator falls out of the PV matmul with no memsets).
  - Device side (per core, SPMD): causal GQA attention for one sequence.
    scores^T [k, q] = K^T-block-stationary matmuls; exp via Act engine with
    fused scale; P^T is directly the PV stationary operand. Scores are
    bank-packed into [128, 1024] PSUM tiles (triple-buffered so neither
    the PE nor the Act engine ever stalls waiting on the other; Act is the
    bottleneck at ~123us of pure exp throughput).
    Causal diag masking via gpsimd affine_select post-exp.
    Output normalized on DVE (reciprocal + broadcast multiply), written
    bf16 and upcast on host.
  - All input staging DMAs are issued upfront on sync (inputs fit SBUF);
    output DMAs follow on the same queue once staging issuance drains.
"""

import numpy as np

B, S, H, KVH, D = 8, 1024, 32, 8, 128
G = H // KVH
NB, BS = 512, 16
T = B * S
SCALE = 0.08838834764831845
NCORES = 8

_compiled = {}


def _build():
    import concourse.bass as bass
    import concourse.bacc as bacc
    import concourse.mybir as mybir
    import concourse.tile as tile

    f32 = mybir.dt.float32
    bf16 = mybir.dt.bfloat16
    EXP = mybir.ActivationFunctionType.Exp

    nc = bacc.Bacc("TRN2", target_bir_lowering=False, debug=False,
                   num_devices=NCORES)
    qtd = nc.dram_tensor("qt", [H * D, S], bf16, kind="ExternalInput").ap()
    ktd = nc.dram_tensor("kt", [KVH * D, S], bf16, kind="ExternalInput").ap()
    vad = nc.dram_tensor("va", [S, KVH * (D + 1)], bf16,
                         kind="ExternalInput").ap()
    od = nc.dram_tensor("out", [S, H * D], bf16, kind="ExternalOutput").ap()

    NT = S // 128            # 8 k/q tiles of 128
    CB = 4                   # q-blocks per chunk (chunk = 512 q cols)
    NCH = NT // CB           # chunks per head

    # Bank-packed score-tile layouts: per chunk, list of tiles; each tile is
    # a list of (k-block j, col offset). Every matmul stays inside a single
    # 512-col (2KB) PSUM bank. Offsets pack the 512/384/256/128-wide blocks
    # with no gaps so one ACTIVATE covers the whole tile.
    PACKS = {
        0: [[(0, 0), (1, 512), (3, 896)],                             # 1024
            [(2, 0)]],                                                # 256
        1: [[(0, 0), (1, 512)],                                       # 1024
            [(2, 0), (3, 512)],                                       # 1024
            [(4, 0), (5, 512), (7, 896)],                             # 1024
            [(6, 0)]],                                                # 256
    }

    with tile.TileContext(nc) as tc:
        with (
            tc.tile_pool(name="ktp", bufs=KVH) as ktp,
            tc.tile_pool(name="qtp", bufs=KVH) as qtp,
            tc.tile_pool(name="vap", bufs=KVH) as vap,
            tc.tile_pool(name="pt", bufs=14) as ptp,
            tc.tile_pool(name="ost", bufs=6) as ostp,
            tc.tile_pool(name="small", bufs=4) as smallp,
            tc.tile_pool(name="psum_s", bufs=3, space="PSUM") as psum_s,
            tc.tile_pool(name="psum_o", bufs=2, space="PSUM") as psum_o,
        ):
            # ---- staging: all upfront, interleaved K/Q/V per group so the
            # first group's operands land first ----
            KTs, QTs, VAs = [], [], []
            for g in range(KVH):
                kt_t = ktp.tile([128, S], bf16, tag="kt")
                nc.sync.dma_start(kt_t[:], ktd[g * 128:(g + 1) * 128, :])
                qt_t = qtp.tile([128, G, S], bf16, tag="qt")
                nc.sync.dma_start(
                    qt_t[:], qtd[g * 512:(g + 1) * 512, :]
                    .rearrange("(h p) s -> p h s", p=128))
                va_t = vap.tile([128, NT, D + 1], bf16, tag="va")
                nc.sync.dma_start(
                    va_t[:], vad[:, g * 129:(g + 1) * 129]
                    .rearrange("(n p) c -> p n c", p=128))
                KTs.append(kt_t)
                QTs.append(qt_t)
                VAs.append(va_t)

            def qk_phase(KT, QT, c):
                # scores^T -> exp -> P^T tiles for one (chunk, head)
                i0 = c * CB
                pts = {}
                for pack in PACKS[c]:
                    st = psum_s.tile([128, 1024], f32, tag="st")
                    w = 0
                    for j, off in pack:
                        n = (CB - max(j - i0, 0)) * 128
                        qcol = max(j, i0) * 128
                        nc.tensor.matmul(
                            st[:, off:off + n],
                            lhsT=KT[:, j * 128:(j + 1) * 128],
                            rhs=QT[:, qcol:qcol + n],
                            start=True, stop=True,
                        )
                        w = max(w, off + n)
                    pt = ptp.tile([128, 1024], bf16, tag="pt")
                    nc.scalar.activation(pt[:, :w], st[:, :w], EXP,
                                         scale=SCALE)
                    for j, off in pack:
                        if j >= i0:
                            # zero strictly-lower (q < k) of diagonal block
                            nc.gpsimd.affine_select(
                                out=pt[:, off:off + 128],
                                in_=pt[:, off:off + 128],
                                compare_op=mybir.AluOpType.is_ge,
                                fill=0.0, base=0,
                                pattern=[[1, 128]],
                                channel_multiplier=-1,
                            )
                        pts[j] = (pt, off)
                return pts

            def pv_phase(VA, pts, ost, c, h4, g):
                # o blocks at col offsets ii*256, width D+1, split into two
                # 1-bank halves so each half frees as soon as its two blocks
                # are normalized; per-bank accumulation groups must not
                # interleave, so each block's start..stop runs to completion.
                i0 = c * CB
                for half in range(2):
                    o = psum_o.tile([128, 512], f32, tag="o")
                    for hi in range(2):
                        ii = half * 2 + hi
                        i = i0 + ii
                        for j in range(i + 1):
                            pt, off = pts[j]
                            col = off + (ii - max(j - i0, 0)) * 128
                            nc.tensor.matmul(
                                o[:, hi * 256: hi * 256 + D + 1],
                                lhsT=pt[:, col:col + 128],
                                rhs=VA[:, j, :],
                                start=(j == 0), stop=(j == i),
                            )
                    rec = smallp.tile([128, 2], f32, tag="rec")
                    nc.vector.reciprocal(rec[:], o[:, D::256])
                    ov = o[:].rearrange("p (b x) -> p b x", x=256)[:, :, 0:D]
                    rbc = (rec[:].rearrange("p b -> p b ()")
                           .broadcast_to((128, 2, D)))
                    nc.vector.tensor_tensor(
                        ost[:, half * 2:half * 2 + 2,
                            h4 * 128:(h4 + 1) * 128], ov, rbc,
                        mybir.AluOpType.mult)
                if h4 == G - 1:
                    nc.sync.dma_start(
                        od[c * 512:(c + 1) * 512, g * 512:(g + 1) * 512]
                        .rearrange("(b p) d -> p b d", p=128),
                        ost[:],
                    )

            # ---- main loop, software-pipelined qk/pv phases ----
            pend = []
            for g in range(KVH):
                for c in range(NCH):
                    ost = ostp.tile([128, CB, G * D], bf16, tag="ost")
                    for h4 in range(G):
                        pts = qk_phase(KTs[g], QTs[g][:, h4, :], c)
                        pend.append((VAs[g], pts, ost, c, h4, g))
                        if len(pend) > 2:
                            pv_phase(*pend.pop(0))
            while pend:
                pv_phase(*pend.pop(0))

    nc.compile()
    return nc


def _get_nc():
    if "nc" not in _compiled:
        _compiled["nc"] = _build()
    return _compiled["nc"]


def kernel(q, k, v, k_cache, v_cache, slot_mapping, block_tables):
    import ml_dtypes
    from concourse.bass_utils import run_bass_kernel_spmd

    bf16 = ml_dtypes.bfloat16
    q = np.ascontiguousarray(np.asarray(q, dtype=np.float32))
    k = np.asarray(k, dtype=np.float32)
    v = np.asarray(v, dtype=np.float32)
    sm = np.asarray(slot_mapping).astype(np.int64)
    bt = np.asarray(block_tables).astype(np.int64)

    # store_kvcache + page gather (reference semantics, pure permutation)
    kc = np.asarray(k_cache, dtype=np.float32).reshape(NB * BS, KVH * D).copy()
    vc = np.asarray(v_cache, dtype=np.float32).reshape(NB * BS, KVH * D).copy()
    kc[sm] = k
    vc[sm] = v
    kg = kc.reshape(NB, BS, KVH * D)[bt].reshape(B, S, KVH * D)
    vg = vc.reshape(NB, BS, KVH * D)[bt].reshape(B, S, KVH * D)

    # per-core device layouts: Q^T, K^T, V+ones, all bf16
    qT = q.reshape(B, S, H * D).transpose(0, 2, 1).astype(bf16)   # [B,H*D,S]
    kT = kg.transpose(0, 2, 1).astype(bf16)                       # [B,KVH*D,S]
    va = np.ones((B, S, KVH, D + 1), dtype=np.float32)
    va[:, :, :, :D] = vg.reshape(B, S, KVH, D)
    va = va.reshape(B, S, KVH * (D + 1)).astype(bf16)

    in_maps = [
        {"qt": np.ascontiguousarray(qT[i]),
         "kt": np.ascontiguousarray(kT[i]),
         "va": np.ascontiguousarray(va[i])}
        for i in range(NCORES)
    ]
    nc = _get_nc()
    res = run_bass_kernel_spmd(nc, in_maps, core_ids=list(range(NCORES)))
    _compiled["last_result"] = res
    out = np.concatenate([res.results[i]["out"] for i in range(NCORES)],
                         axis=0)
    return out.astype(np.float32)
